# revision 12
# baseline (speedup 1.0000x reference)
"""Trainium2 Bass kernel for nn_Decoder_16690242913225.

kernel(**inputs) takes the FULL (unsharded) inputs (B=512) and returns the
full (512, 64, 256) float32 output.  Internally the batch dim is sharded
8 ways (64 rows per NeuronCore, pure data parallelism) and one SPMD Bass
program runs on cores 0-7.

Host-side prep (prep_neff_inputs) does everything cheap that the PE would
otherwise burn columns on: weights are pre-transposed and pre-cast to
bf16, the teacher-forcing shift is applied to tosT, the encoder memories
are shipped in both column-major (for scores) and row-major (for a@enc)
bf16 layouts, and — because the attention here is single-head linear —
the q/k and v/o projections are FOLDED into single matrices
  m  = wq^T wk / sqrt(E)     (scores = (h m) enc^T)
  w2 = wv^T wo^T             (out    = (a enc) w2)
so the device never computes k or v at all.  Per-core NEFF input drops
from 30.7 MB fp32 to ~17.7 MB bf16.

Device program (build_decoder):
  P1  64-step LSTM: per step the x-part (tosT stationary, w_ihT moving)
      and h-part (h^T stationary, w_hhT moving) accumulate the 4 gate
      quarters in PSUM; nonlinearities on ACT/DVE in f32; h (bf16) is
      PE-transposed into hsT which is both the next step's stationary and
      the q'-projection input.
  P2  q' = h @ m (scale folded in), stored bf16 as qT [e, (t b)].
  P3  per-b attention directly against the encoder: scores = q'_b
      (stationary, strided slice) x encT_b; exp without max-subtraction
      (|scores| < 2 here); softmax normalization folded into a as a
      per-partition ACT scale; o^T = enc_b^T a^T via stationary=enc rows.
  P4  out-proj with w2 -> catT (bf16, spilled to DRAM).
  P5  logits^T = out_wT^T cat, scaled 0.5 into zT.
  P6  PE-transpose zT to row-major tiles.
  P7  entmax15: tau by 5 Newton iterations on f(t) = sum(relu(z-t)^2)-1
      (validated against the sort-based reference), y = relu(z-tau)^2.

The neuronxcc walrus in this container rejects instructions carrying
more than one embedded sem wait, so excess waits are moved onto
same-engine NoOps (in-order queues make this equivalent).
"""

import sys

sys.path.insert(0, "/opt/trn_rl_repo")

from contextlib import ExitStack

import numpy as np

import bass_rust
import concourse.bass as bass
import concourse.tile as tile
from concourse import mybir
from concourse.masks import make_identity
from concourse.vector_clock import ScopedClock, VectorClock

F32 = mybir.dt.float32
BF16 = mybir.dt.bfloat16
AF = mybir.ActivationFunctionType
OP = mybir.AluOpType
AX = mybir.AxisListType

N_CORES = 8
B, T, E, V = 64, 64, 512, 256     # per-core batch, seq, embed, vocab
G = 4 * E
KE, KV = E // 128, V // 128
SC, ST = 64, 16
NR = T * B

# every NEFF input is shipped pre-sharded: axis 0 is the 8-core concat
BATCH_KEYS = (
    "tosT", "w_ihT", "w_hhT", "h0T", "c0",
    "m_c", "m_t", "u_c", "u_t",
    "encT_c", "encR_c", "encT_t", "encR_t",
)

# ---------------------------------------------------------------------------
# Workarounds for the 1-wait-per-instruction walrus limit
# ---------------------------------------------------------------------------


def _patched_drain_and_barrier(self, tick_clock, wait_clock):
    gc = tick_clock.global_clock
    n = len(gc)
    for i in range(n):
        if gc[i] == 0:
            continue
        vec = [0] * n
        vec[i] = gc[i]
        nop = self.nc.sync.nop(nofuse=True, hint="drain_wait_split")
        wait_clock.add_sem_waits(nop.ins, ScopedClock({None: VectorClock(vec)}))
    self.nc.sync.drain()
    self.nc.all_engine_barrier()
    assert self.sems is not None
    popped = self.nc._tile_sem_poison_stack.pop()
    assert popped is self._sem_poison
    self.nc.clear_and_free_semaphores(list(self.sems.allocated().values()))
    self.nc.all_engine_barrier()


tile.TileContext._drain_and_barrier = _patched_drain_and_barrier

_nop_counter = [0]


def split_multi_waits(nc, max_waits=1):
    """Move excess sem waits from any instruction onto same-engine NoOps
    inserted immediately before it (engine queues are in-order, so the
    blocking semantics are identical)."""
    for f in nc.m.functions:
        for blk in f.blocks:
            insts = blk.instructions
            new = []
            changed = False
            for inst in insts:
                si = inst.sync_info
                if si is not None and si.on_wait and len(si.on_wait) > max_waits:
                    waits = list(si.on_wait)
                    for w in waits[:-max_waits]:
                        _nop_counter[0] += 1
                        nop = mybir.InstNoOp(
                            name=f"wsplit_{_nop_counter[0]}", ins=[], outs=[])
                        nop.engine = inst.engine
                        nop.sync_info = bass_rust.SyncInfo(on_wait=[w], on_update=[])
                        new.append(nop)
                    inst.sync_info = bass_rust.SyncInfo(
                        on_wait=waits[-max_waits:],
                        on_update=list(si.on_update or []))
                    changed = True
                new.append(inst)
            if changed:
                blk.instructions = new


# ---------------------------------------------------------------------------
# Kernel program
# ---------------------------------------------------------------------------


def build_decoder(nc, NEWTON=5):
    NCH = 512

    din = {}
    for name, shape in [
        ("tosT", [V, NR]), ("w_ihT", [V, G]), ("w_hhT", [E, G]),
        ("h0T", [E, B]), ("c0", [B, E]),
        ("m_c", [E, E]), ("m_t", [E, E]),
        ("u_c", [E, V]), ("u_t", [E, V]),
        ("encT_c", [E, B * SC]), ("encR_c", [B * SC, E]),
        ("encT_t", [E, B * ST]), ("encR_t", [B * ST, E]),
    ]:
        dt = F32 if name == "c0" else BF16
        din[name] = nc.dram_tensor(name, shape, dt, kind="ExternalInput").ap()
    out = nc.dram_tensor("out", [B, T, V], F32, kind="ExternalOutput").ap()
    out_tbv = out.rearrange("b t v -> t b v")

    with tile.TileContext(nc) as tc:
        es = ExitStack()
        const = es.enter_context(tc.tile_pool(name="const", bufs=1))
        dramp = es.enter_context(tc.tile_pool(name="dramp", bufs=1, space="DRAM"))

        ident_f32 = const.tile([128, 128], F32, tag="ident_f32", name="ident_f32")
        make_identity(nc, ident_f32)
        ident_bf16 = const.tile([128, 128], BF16, tag="ident_bf16", name="ident_bf16")
        nc.vector.tensor_copy(out=ident_bf16, in_=ident_f32)
        zeros_row = const.tile([128, V], F32, tag="zeros_row", name="zeros_row")
        nc.vector.memset(zeros_row, 0.0)

        def transpose_into(pool, dst, src, ident, ptag="tp"):
            pt = pool.tile([128, 128], src.dtype, tag=ptag, name=ptag)
            pt = pt[: src.shape[-1], : src.shape[0]]
            nc.tensor.transpose(pt, src, ident[: src.shape[0], : src.shape[0]])
            nc.vector.tensor_copy(out=dst, in_=pt)

        # =========== P0: DMA loads (no transposes — host pre-transposed) ====
        es_w = ExitStack()
        wl = es_w.enter_context(tc.tile_pool(name="wl", bufs=1))
        es_hsT = ExitStack()
        hp = es_hsT.enter_context(tc.tile_pool(name="hsT", bufs=1, side="right"))
        hsT = [hp.tile([128, NR], BF16, tag=f"hsT{k}", name=f"hsT{k}")
               for k in range(KE)]

        w_ihT = [wl.tile([128, G], BF16, tag=f"w_ihT{k}", name=f"w_ihT{k}")
                 for k in range(KV)]
        w_hhT = [wl.tile([128, G], BF16, tag=f"w_hhT{k}", name=f"w_hhT{k}")
                 for k in range(KE)]
        tosT = [wl.tile([128, NR], BF16, tag=f"tosT{k}", name=f"tosT{k}")
                for k in range(KV)]
        h0T = [wl.tile([128, B], BF16, tag=f"h0T{k}", name=f"h0T{k}")
               for k in range(KE)]
        for k in range(KV):
            nc.sync.dma_start(w_ihT[k], din["w_ihT"][k * 128:(k + 1) * 128, :])
            nc.sync.dma_start(tosT[k], din["tosT"][k * 128:(k + 1) * 128, :])
        for k in range(KE):
            nc.sync.dma_start(w_hhT[k], din["w_hhT"][k * 128:(k + 1) * 128, :])
            nc.sync.dma_start(h0T[k], din["h0T"][k * 128:(k + 1) * 128, :])
        c0 = wl.tile([B, E], F32, tag="c0", name="c0")
        nc.sync.dma_start(c0, din["c0"])

        # =========== P1: LSTM ===========
        es_lstm = ExitStack()
        lw = es_lstm.enter_context(tc.tile_pool(name="lstm_work", bufs=2))
        cpool = es_lstm.enter_context(tc.tile_pool(name="cpool", bufs=2))
        ps1 = es_lstm.enter_context(tc.tile_pool(name="ps1", bufs=2, space="PSUM"))

        c_cur = c0
        h_prev = None

        for t in range(T):
            # i,g quarters are consumed early -> single buffer; f,o double.
            p_ig = ps1.tile([64, 1024], F32, tag="p_ig", name="p_ig", bufs=1)
            p_fo = ps1.tile([64, 1024], F32, tag="p_fo", name="p_fo", bufs=2)

            def quarter(qi):
                # gate order in memory: i, f, g, o
                return (p_ig, slice(0, 512)) if qi == 0 else (
                    p_fo, slice(0, 512)) if qi == 1 else (
                    p_ig, slice(512, 1024)) if qi == 2 else (
                    p_fo, slice(512, 1024))

            # x-part first: no dependence on h_{t-1}, fills the PE while the
            # previous step's tail finishes.  tosT col block t is the
            # teacher-forced input (host pre-shifted; t=0 block is zeros).
            for qi in (0, 2, 1, 3):
                pg, sl = quarter(qi)
                wsl = slice(qi * 512, (qi + 1) * 512)
                for kx in range(KV):
                    nc.tensor.matmul(pg[:, sl], tosT[kx][:, t * B:(t + 1) * B],
                                     w_ihT[kx][:, wsl],
                                     start=(kx == 0), stop=False,
                                     skip_group_check=True)
            if t > 0:
                for k in range(KE):
                    transpose_into(ps1, hsT[k][:, (t - 1) * B: t * B],
                                   h_prev[:, k * 128:(k + 1) * 128], ident_bf16,
                                   ptag="hT")
                h_stat = [hsT[k][:, (t - 1) * B: t * B] for k in range(KE)]
            else:
                h_stat = [h0T[k][:] for k in range(KE)]

            # h-part quarter-outer in (i, g, f, o) order: each gate's
            # nonlinearity starts while later quarters still stream.
            for qi in (0, 2, 1, 3):
                pg, sl = quarter(qi)
                wsl = slice(qi * 512, (qi + 1) * 512)
                for k in range(KE):
                    nc.tensor.matmul(pg[:, sl], h_stat[k],
                                     w_hhT[k][:, wsl],
                                     start=False, stop=(k == KE - 1),
                                     skip_group_check=True)

            si = lw.tile([64, 512], F32, tag="si", name="si")
            nc.scalar.activation(si, p_ig[:, 0:512], AF.Sigmoid)
            tg = lw.tile([64, 512], F32, tag="tg", name="tg")
            nc.scalar.activation(tg, p_ig[:, 512:1024], AF.Tanh)
            sf = lw.tile([64, 512], F32, tag="sf", name="sf")
            nc.scalar.activation(sf, p_fo[:, 0:512], AF.Sigmoid)
            so = lw.tile([64, 512], F32, tag="so", name="so")
            nc.scalar.activation(so, p_fo[:, 512:1024], AF.Sigmoid)
            m1 = lw.tile([64, 512], F32, tag="m1", name="m1")
            nc.vector.tensor_tensor(m1, si, tg, OP.mult)
            fc = lw.tile([64, 512], F32, tag="fc", name="fc")
            nc.vector.tensor_tensor(fc, sf, c_cur, OP.mult)
            c_next = cpool.tile([B, E], F32, tag="c", name="c")
            nc.vector.tensor_tensor(c_next, fc, m1, OP.add)
            tcs = lw.tile([64, 512], F32, tag="tc", name="tc")
            nc.scalar.activation(tcs, c_next, AF.Tanh)
            h_t = lw.tile([64, 512], BF16, tag="h", name="h")
            nc.vector.tensor_tensor(h_t, so, tcs, OP.mult)
            h_prev, c_cur = h_t, c_next

        for k in range(KE):
            transpose_into(ps1, hsT[k][:, (T - 1) * B: T * B],
                           h_prev[:, k * 128:(k + 1) * 128], ident_bf16,
                           ptag="hT")

        es_lstm.close()
        es_w.close()

        # =========== P2: q' projections (scale folded into m) ===========
        es_mw = ExitStack()
        mw = es_mw.enter_context(tc.tile_pool(name="mha_w", bufs=1))
        es_oT = ExitStack()
        op_ = es_oT.enter_context(tc.tile_pool(name="oTp", bufs=1))
        oT_all = {w: [op_.tile([128, NR], BF16, tag=f"oT{w}{k}", name=f"oT{w}{k}")
                      for k in range(KE)] for w in ("c", "t")}
        es_qT = ExitStack()
        qpool = es_qT.enter_context(tc.tile_pool(name="qT", bufs=1))
        es_p2 = ExitStack()
        ps2q = es_p2.enter_context(tc.tile_pool(name="ps2q", bufs=2, space="PSUM"))

        def load_w(name, tag):
            tiles = [mw.tile([128, E], BF16, tag=f"{tag}{k}", name=f"{tag}{k}")
                     for k in range(KE)]
            for k in range(KE):
                nc.sync.dma_start(tiles[k], din[name][k * 128:(k + 1) * 128, :])
            return tiles

        mT = {"c": load_w("m_c", "mc"), "t": load_w("m_t", "mt")}

        qT = {}
        for which in ("c", "t"):
            qT[which] = [qpool.tile([128, NR], BF16, tag=f"qT{which}{m}",
                                    name=f"qT{which}{m}") for m in range(KE)]
            for m in range(KE):
                for n in range(NR // NCH):
                    pq = ps2q.tile([128, NCH], F32, tag="qp", name="qp")
                    for k in range(KE):
                        nc.tensor.matmul(pq, mT[which][k][:, m * 128:(m + 1) * 128],
                                         hsT[k][:, n * NCH:(n + 1) * NCH],
                                         start=(k == 0), stop=(k == KE - 1))
                    nc.scalar.copy(qT[which][m][:, n * NCH:(n + 1) * NCH], pq)
        es_p2.close()
        es_hsT.close()

        def load_u(name, tag):
            tiles = [mw.tile([128, V], BF16, tag=f"{tag}{k}", name=f"{tag}{k}")
                     for k in range(KE)]
            for k in range(KE):
                nc.sync.dma_start(tiles[k], din[name][k * 128:(k + 1) * 128, :])
            return tiles

        uT = {"c": load_u("u_c", "uc"), "t": load_u("u_t", "ut")}

        # =========== P3: attention (k/v and out-proj folded away) ===========
        for which, S, encT_d, encR_d in [
            ("c", SC, din["encT_c"], din["encR_c"]),
            ("t", ST, din["encT_t"], din["encR_t"]),
        ]:
            es_att = ExitStack()
            ap_ = es_att.enter_context(tc.tile_pool(name=f"att{which}", bufs=3))
            ep_ = es_att.enter_context(tc.tile_pool(name=f"encp{which}", bufs=1))
            oT = oT_all[which]
            encT = [ep_.tile([128, B * S], BF16, tag=f"encT{k}", name=f"encT{k}")
                    for k in range(KE)]
            for k in range(KE):
                nc.sync.dma_start(encT[k], encT_d[k * 128:(k + 1) * 128, :])
            es_ps3 = ExitStack()
            ps3s = es_ps3.enter_context(tc.tile_pool(name="ps3s", bufs=2,
                                                     space="PSUM"))
            encR_b = encR_d.rearrange("(b s) e -> b s e", s=S)

            for b in range(B):
                vb = ap_.tile([S, E], BF16, tag="vb", name="vb", bufs=4)
                nc.sync.dma_start(vb, encR_b[b])
                p_s = ps3s.tile([T, S], F32, tag="p_s", name="p_s", bufs=3)
                for k in range(KE):
                    qslice = qT[which][k][:].rearrange(
                        "p (t b) -> p t b", b=B)[:, :, b]
                    nc.tensor.matmul(p_s, qslice, encT[k][:, b * S:(b + 1) * S],
                                     start=(k == 0), stop=(k == KE - 1))
                exps = ap_.tile([T, S], BF16, tag="exps", name="exps")
                sume = ap_.tile([T, 1], F32, tag="sume", name="sume")
                nc.scalar.activation(exps, p_s, AF.Exp, accum_out=sume)
                r = ap_.tile([T, 1], F32, tag="recip", name="recip")
                nc.vector.reciprocal(r, sume)
                # fold softmax normalization into a (per-partition over t)
                expsn = ap_.tile([T, S], BF16, tag="expsn", name="expsn")
                nc.scalar.activation(expsn, exps, AF.Copy, scale=r)
                p_aT = ps3s.tile([S, T], BF16, tag="p_aT", name="p_aT", bufs=2)
                nc.tensor.transpose(p_aT, expsn, ident_bf16[:T, :T])
                aT = ap_.tile([S, T], BF16, tag="aT", name="aT")
                nc.scalar.copy(aT, p_aT)
                p_ot = ps3s.tile([128, KE * T], F32, tag="p_ot", name="p_ot",
                                 bufs=2)
                for k in range(KE):
                    nc.tensor.matmul(p_ot[:, k * T:(k + 1) * T],
                                     vb[:, k * 128:(k + 1) * 128], aT,
                                     start=True, stop=True,
                                     skip_group_check=True)
                for k in range(KE):
                    oTv = oT[k][:].rearrange("p (t b) -> p t b", b=B)
                    if k == 0:
                        nc.scalar.copy(oTv[:, :, b], p_ot[:, k * T:(k + 1) * T])
                    else:
                        nc.vector.tensor_copy(out=oTv[:, :, b],
                                              in_=p_ot[:, k * T:(k + 1) * T])
            es_ps3.close()
            es_att.close()
        es_qT.close()

        # =========== P5: logits^T (scaled 0.5) ===========
        es_z = ExitStack()
        zp = es_z.enter_context(tc.tile_pool(name="zp", bufs=1, side="right"))
        zT = [zp.tile([128, NR], F32, tag=f"zT{m}", name=f"zT{m}") for m in range(KV)]
        es_p5 = ExitStack()
        ps5 = es_p5.enter_context(tc.tile_pool(name="ps5", bufs=2, space="PSUM"))
        for n in range(NR // NCH):
            for m in range(KV):
                pl = ps5.tile([128, NCH], F32, tag="lp", name="lp")
                for wi, w in enumerate(("c", "t")):
                    for k in range(KE):
                        nc.tensor.matmul(pl, uT[w][k][:, m * 128:(m + 1) * 128],
                                         oT_all[w][k][:, n * NCH:(n + 1) * NCH],
                                         start=(wi == 0 and k == 0),
                                         stop=(wi == 1 and k == KE - 1))
                nc.scalar.copy(zT[m][:, n * NCH:(n + 1) * NCH], pl)
        es_p5.close()

        # =========== P6/P7: transpose + entmax ===========
        es_e = ExitStack()
        ep = es_e.enter_context(tc.tile_pool(name="entmax", bufs=2))
        zrows = es_e.enter_context(tc.tile_pool(name="zrows", bufs=1))
        ps6 = es_e.enter_context(tc.tile_pool(name="ps6", bufs=2, space="PSUM"))
        NT = NR // 128
        NG = min(4, NT)          # independent Newton groups: group g's
        GT = NT // NG            # iterations overlap later groups' transposes
        for grp in range(NG):
            tiles = range(grp * GT, (grp + 1) * GT)
            ztiles = {}
            negt = zrows.tile([128, GT], F32, tag=f"negt{grp}_0",
                              name=f"negt{grp}_0")
            for i in tiles:
                zh = zrows.tile([128, V], F32, tag=f"zh{i}", name=f"zh{i}")
                for m in range(KV):
                    transpose_into(ps6, zh[:, m * 128:(m + 1) * 128],
                                   zT[m][:, i * 128:(i + 1) * 128], ident_f32)
                ztiles[i] = zh
                c_ = i - grp * GT
                zmax = ep.tile([128, 1], F32, tag="zmax", name="zmax")
                nc.vector.tensor_reduce(zmax, zh, axis=AX.X, op=OP.max)
                nc.vector.tensor_scalar(out=negt[:, c_:c_ + 1], in0=zmax,
                                        scalar1=-1.0, scalar2=1.0,
                                        op0=OP.mult, op1=OP.add)

            for it in range(NEWTON):
                su = zrows.tile([128, GT], F32, tag=f"su{grp}_{it}",
                                name=f"su{grp}_{it}")
                su2 = zrows.tile([128, GT], F32, tag=f"su2{grp}_{it}",
                                 name=f"su2{grp}_{it}")
                for i in tiles:
                    c_ = i - grp * GT
                    u = ep.tile([128, V], F32, tag="u", name="u")
                    nc.vector.scalar_tensor_tensor(
                        out=u, in0=ztiles[i], scalar=negt[:, c_:c_ + 1],
                        in1=zeros_row, op0=OP.add, op1=OP.max,
                        accum_out=su[:, c_:c_ + 1])
                    u2 = ep.tile([128, V], F32, tag="u2", name="u2")
                    nc.scalar.activation(u2, u, AF.Square,
                                         accum_out=su2[:, c_:c_ + 1])
                rr = ep.tile([128, GT], F32, tag="rr", name="rr")
                nc.vector.reciprocal(rr, su)
                d = ep.tile([128, GT], F32, tag="d", name="d")
                nc.vector.tensor_scalar(out=d, in0=su2, scalar1=1.0, scalar2=0.5,
                                        op0=OP.subtract, op1=OP.mult)
                e_ = ep.tile([128, GT], F32, tag="e_", name="e_")
                nc.vector.tensor_tensor(e_, d, rr, OP.mult)
                negt2 = zrows.tile([128, GT], F32, tag=f"negt{grp}_{it + 1}",
                                   name=f"negt{grp}_{it + 1}")
                nc.vector.tensor_tensor(negt2, negt, e_, OP.subtract)
                negt = negt2

            for i in tiles:
                c_ = i - grp * GT
                u = ep.tile([128, V], F32, tag="u", name="u")
                nc.vector.scalar_tensor_tensor(
                    out=u, in0=ztiles[i], scalar=negt[:, c_:c_ + 1],
                    in1=zeros_row, op0=OP.add, op1=OP.max)
                y = ep.tile([128, V], F32, tag="y", name="y")
                nc.scalar.activation(y, u, AF.Square)
                t0 = (i * 128) // B
                for j in range(2):
                    nc.sync.dma_start(out_tbv[t0 + j], y[j * 64:(j + 1) * 64, :])
        es_z.close()
        es_e.close()
        es_oT.close()
        es_mw.close()
        es.close()
    return nc


_CACHE = {}


def _get_nc():
    if "nc" not in _CACHE:
        nc = bass.Bass("TRN2", target_bir_lowering=False, debug=False, num_devices=1)
        build_decoder(nc)
        split_multi_waits(nc)
        _CACHE["nc"] = nc
    return _CACHE["nc"]


def prep_neff_inputs(inputs):
    """Full-batch host arrays keyed by NEFF input name.  Axis 0 of every
    array is the 8-core concat (weights repeated per core); slicing
    [c*d0:(c+1)*d0] yields core c's input."""
    import ml_dtypes
    bf16 = ml_dtypes.bfloat16

    f = {k: np.asarray(v, dtype=np.float32) for k, v in inputs.items()}
    Bfull = f["char_encoding"].shape[0]
    nb = Bfull // N_CORES
    QS = 1.0 / np.sqrt(np.float32(E))

    # per-core batch data
    tos = f["true_output_seq"]                          # (512, T, V)
    xs = np.concatenate([np.zeros_like(tos[:, :1]), tos[:, 1:]], axis=1)
    h0 = np.concatenate([f["char_hn0"], f["tag_hn0"]], axis=-1)   # (512, E)
    c0 = np.concatenate([f["char_cn0"], f["tag_cn0"]], axis=-1)

    def per_core_batch(fn):
        return np.ascontiguousarray(np.concatenate(
            [fn(c) for c in range(N_CORES)], axis=0))

    def rep_weight(w):
        return np.ascontiguousarray(np.concatenate([w] * N_CORES, axis=0))

    out = {}
    # tosT: [V, (t b)] per core
    out["tosT"] = per_core_batch(
        lambda c: xs[c * nb:(c + 1) * nb].transpose(2, 1, 0).reshape(V, T * nb)
    ).astype(bf16)
    out["h0T"] = per_core_batch(
        lambda c: h0[c * nb:(c + 1) * nb].T).astype(bf16)
    out["c0"] = per_core_batch(lambda c: c0[c * nb:(c + 1) * nb])
    for which, enc_key, S in (("c", "char_encoding", SC), ("t", "tag_encoding", ST)):
        enc = f[enc_key]
        out[f"encT_{which}"] = per_core_batch(
            lambda c: enc[c * nb:(c + 1) * nb].transpose(2, 0, 1).reshape(E, nb * S)
        ).astype(bf16)
        out[f"encR_{which}"] = per_core_batch(
            lambda c: enc[c * nb:(c + 1) * nb].reshape(nb * S, E)).astype(bf16)

    # weights (identical per core)
    out["w_ihT"] = rep_weight(f["w_ih"].T).astype(bf16)
    out["w_hhT"] = rep_weight(f["w_hh"].T).astype(bf16)
    out["m_c"] = rep_weight(f["char_wq"].T @ f["char_wk"] * QS).astype(bf16)
    out["m_t"] = rep_weight(f["tag_wq"].T @ f["tag_wk"] * QS).astype(bf16)
    ow = f["out_w"]
    out["u_c"] = rep_weight(
        f["char_wv"].T @ f["char_wo"].T @ ow[:, :E].T * 0.5).astype(bf16)
    out["u_t"] = rep_weight(
        f["tag_wv"].T @ f["tag_wo"].T @ ow[:, E:].T * 0.5).astype(bf16)
    return out


def postprocess_neff_out(arr):
    """Concatenated (512, ...) NEFF output -> (512, 64, 256) float32."""
    return np.asarray(arr, dtype=np.float32)


def kernel(**inputs):
    from concourse.bass_utils import run_bass_kernel_spmd

    for bias in ("b_ih", "b_hh", "char_bq", "char_bk", "char_bv", "char_bo",
                 "tag_bq", "tag_bk", "tag_bv", "tag_bo", "out_b"):
        if bias in inputs and np.any(np.asarray(inputs[bias])):
            raise NotImplementedError(f"nonzero bias {bias} not supported")

    nc = _get_nc()
    full = prep_neff_inputs(inputs)
    in_maps = []
    for c in range(N_CORES):
        m = {}
        for k, v in full.items():
            d0 = v.shape[0] // N_CORES
            m[k] = v[c * d0:(c + 1) * d0]
        in_maps.append(m)
    res = run_bass_kernel_spmd(nc, in_maps, core_ids=list(range(N_CORES)))
    return postprocess_neff_out(
        np.concatenate([res.results[c]["out"] for c in range(N_CORES)], axis=0))
